# revision 41
# baseline (speedup 1.0000x reference)
"""Trainium2 Bass kernel for nn_Diffusion_29789893165499 (gnn_message_passing).

Full inputs in, full output out. Shards electrons (and hence edges) across
8 NeuronCores; each core computes its 128 electrons' message passing +
dense tail locally. No cross-core communication.

Per-core math (electron i, nucleus k, edge-feat j, out-dim d):
  P[(il,j), d] = sum_k E[i,k,j] * T[k,d]       bf16 PE, E stationary,
                                               full 256-deep k contraction
  praw = bf16(P)                               Act evict (most chunks)
  pm = praw * W_edge[j,d]                      DVE 2x bf16 multiply
                                               (direct chunks: DVE 1x
                                                straight from PSUM)
  hT[d, i]  = out0T[d, i]                      identity matmul seeds PSUM
            + sum_{(il,j)} pm * seln[i]        PE "selector" matmuls, d-major
  out0T = ((elec @ W_out + b_out) * s2)^T      precomputed on host (1.2% of
                                               the FLOPs), shipped bf16
  h1 = silu(hT)                                Act Silu (GAIN folded in W2)
  y = h1 @ (GAIN*W_out2)                       PE, d-major quads
  z = silu(y + b2)                             Act Silu w/ per-partition bias
  fin = z*K2 + elecT (elec/sqrt2 prefolded)    DVE scalar_tensor_tensor
Everything d-major after the selector: no transposes anywhere.

Chunks of 4 blocks early, 2 blocks late: the final chunks' PSUM->SBUF
readout (the serial end-game) is half-size.  All heavy DMA rides the sync
queue; wrep/seln/idn and the tail constants ride the scalar queue so their
HWDGE-generation slots interleave with the first edge chunks.
"""
import sys

if "/opt/trn_rl_repo" not in sys.path:
    sys.path.insert(0, "/opt/trn_rl_repo")

import numpy as np
import ml_dtypes

BF16 = ml_dtypes.bfloat16

N_CORES = 8
N_EL, N_NUC, DIM, EDIM = 1024, 256, 256, 32
NI = N_EL // N_CORES          # 128 electrons per core
NE = NI * N_NUC               # 32768 edges per core
NB = NI // 4                  # 32 blocks of 4 electrons
CHUNKS = (4, 4, 4, 4, 4, 4, 2, 2, 2, 2)      # blocks per chunk
EVICT = frozenset({1, 2, 3, 4, 6, 7, 8, 9})  # chunks on the Act-evict path
SEL_LAG = 3                   # selector trails P-matmuls by 3 chunks
_CUM = tuple(np.cumsum((0,) + CHUNKS))       # block offsets per chunk

# packed bf16 const layout (columns).  tq rides at the head of the edges
# tensor (same kh-major row structure) so chunk 0 needs only one DMA.
_EDGE0 = 256                  # edge columns start after tq in the edges dram
_OFF_WREP = 0                 # 256
_OFF_SELN = 256               # 128
_OFF_IDN = 384                # 128 (identity matrix)
_OFF_OUT0 = 512               # 2 x 128 (host-precomputed out0^T, d-major)
_OFF_ELECT = 768              # 2 x 128 (elec^T / sqrt2)
_OFF_W2Q = 1024               # 2 x 256
_OFF_B2 = 1536                # 2 (b_out2 halves as per-partition cols)
_CB_COLS = 1538

_s = np.random.default_rng(0).standard_normal(1 << 20).astype(np.float32)
GAIN = float(1.0 / (_s / (1.0 + np.exp(-_s))).std())
INV_SQRT2 = float(1.0 / np.sqrt(2.0))
K2 = GAIN * INV_SQRT2

_RUNNER = None


def _build_nc(reps=None, stage=99, chunks=CHUNKS, evict=EVICT, warmn=7,
              split3=True, tail_mode=0, use_b2=False):
    """Build the per-core Bass module. reps!=None wraps the main body in a
    device-side For_i loop (for wall-clock slope timing only)."""
    import concourse.bacc as bacc
    import concourse.mybir as mybir
    from concourse.tile import TileContext
    from concourse.bass import AP

    f32 = mybir.dt.float32
    bf16 = mybir.dt.bfloat16
    AF = mybir.ActivationFunctionType
    ALU = mybir.AluOpType
    nchunk = len(chunks)
    cum = tuple(int(x) for x in np.cumsum((0,) + tuple(chunks)))
    assert cum[-1] == NB

    nc = bacc.Bacc("TRN2")
    edges = nc.dram_tensor("edges", [2 * 128, _EDGE0 + NB * 128], bf16,
                           kind="ExternalInput")
    cb = nc.dram_tensor("cb", [128, _CB_COLS], bf16, kind="ExternalInput")
    # output ships bf16 (one final rounding, ~0.2% rel; upcast on host):
    # halves the closing store's payload on the critical DMA chain
    out = nc.dram_tensor("out", [DIM, NI], bf16, kind="ExternalOutput")

    edges_v = edges.rearrange("(kh p) f -> p kh f", kh=2)   # [128, 2, 256+4096]

    with TileContext(nc) as tc:
        with tc.tile_pool(name="const", bufs=1) as const, \
             tc.tile_pool(name="ebuf", bufs=len(chunks)) as ebuf, \
             tc.tile_pool(name="praw", bufs=3) as prawp, \
             tc.tile_pool(name="pmb", bufs=5) as pmb, \
             tc.tile_pool(name="work", bufs=2) as work, \
             tc.tile_pool(name="pp", bufs=3, space="PSUM") as pp, \
             tc.tile_pool(name="pacc", bufs=2, space="PSUM") as pacc:

            # ---- constants (outside the timing loop) ----
            cb_t = const.tile([128, _CB_COLS], bf16, tag="cb")
            nc.scalar.dma_start(out=cb_t[:, _OFF_WREP:_OFF_OUT0],
                                in_=cb[:, _OFF_WREP:_OFF_OUT0])
            nc.scalar.dma_start(out=cb_t[:, _OFF_OUT0:_OFF_ELECT],
                                in_=cb[:, _OFF_OUT0:_OFF_ELECT])
            # elecT/w2q/b2 are tail-only (first use ~11us): their DMA is
            # emitted inside body() after the edge stream so their payload
            # doesn't delay the edge chunks on the serial DMA engines.

            def w2q_t(kh):
                return cb_t[:, _OFF_W2Q + 256 * kh:_OFF_W2Q + 256 * (kh + 1)]

            out0_t = cb_t[:, _OFF_OUT0:_OFF_OUT0 + 256]
            elecT_t = cb_t[:, _OFF_ELECT:_OFF_ELECT + 256]
            seln_t = cb_t[:, _OFF_SELN:_OFF_SELN + 128]
            idn_t = cb_t[:, _OFF_IDN:_OFF_IDN + 128]
            wrep_ap = cb_t[:, _OFF_WREP:_OFF_WREP + 256]
            b2_col = [cb_t[:, _OFF_B2 + dh:_OFF_B2 + dh + 1] for dh in range(2)]
            # pin the silu act table before any Act work (Copy/Silu share a
            # table set)
            warm = const.tile([128, 1], f32, tag="warm")
            nc.scalar.activation(warm[:], cb_t[:, 0:1], AF.Silu)

            # PE p-state warmup: filler matmuls sized to bridge from program
            # start until the first edge chunk (plus a little banked data) is
            # in SBUF, so the P-matmul stream never idles afterwards.
            scfill = const.tile([128, 512], bf16, tag="scfill")
            nc.gpsimd.memset(scfill[:], 1.0)
            fps = pp.tile([128, 1024], f32, tag="pc", name="fill")
            for _ in range(warmn):
                nc.tensor.matmul(fps[:, 0:512], scfill[:, 0:128], scfill[:],
                                 start=True, stop=True, skip_group_check=True)
            fcons = const.tile([128, 1], f32, tag="fcons")
            nc.vector.tensor_copy(fcons[:], fps[:, 0:1])

            def body():
                hy = pacc.tile([128, 2 * DIM], f32, tag="hy")
                hacc = hy[:, 0:DIM]
                yps = hy[:, DIM:2 * DIM]

                ets, pms = {}, {}

                # ---- edge DMAs: one per chunk, all queued up front.
                # Chunk 0's DMA also carries tq (the leading _EDGE0 columns
                # of the edges tensor), so the first P-matmul waits on a
                # single transfer.
                for c in range(nchunk):
                    pre = _EDGE0 if c == 0 else 0
                    w = pre + 128 * chunks[c]
                    et = ebuf.tile([128, 2 * w], bf16, tag="e", name=f"e{c}")
                    nc.sync.dma_start(
                        out=et[:].rearrange("p (kh f) -> p kh f", kh=2),
                        in_=edges_v[:, :, _EDGE0 + 128 * cum[c] - pre:
                                    _EDGE0 + 128 * cum[c + 1]])
                    ets[c] = et
                nc.sync.dma_start(out=cb_t[:, _OFF_ELECT:_CB_COLS],
                                  in_=cb[:, _OFF_ELECT:_CB_COLS])
                et0w = _EDGE0 + 128 * chunks[0]

                def tq_t(kh):
                    return ets[0][:, et0w * kh:et0w * kh + _EDGE0]

                def p_mms(c):
                    # P[(il,j), d] for the blocks of chunk c
                    et = ets[c]
                    nbl = chunks[c]
                    pre = _EDGE0 if c == 0 else 0
                    w = pre + 128 * nbl
                    pc = pp.tile([128, nbl * DIM], f32, tag="pc",
                                 name=f"pc{c}")
                    for b8 in range(nbl):
                        eo = pre + 128 * b8
                        for kh in range(2):
                            nc.tensor.matmul(
                                pc[:, DIM * b8:DIM * (b8 + 1)],
                                et[:, w * kh + eo:w * kh + eo + 128],
                                tq_t(kh),
                                start=(kh == 0), stop=(kh == 1))
                    return pc

                def emit_wmult(c, pc, pool_mult=frozenset()):
                    # pm = P * W_edge (broadcast per block).  Evict chunks:
                    # Act copy f32->bf16 then DVE 2x multiply (Pool handles
                    # a couple of slack multiplies to unload DVE); direct
                    # chunks: DVE 1x straight from PSUM, in halves so the
                    # first half overlaps the second half's P-matmuls.
                    nbl = chunks[c]
                    pm = pmb.tile([128, nbl * DIM], bf16, tag="pm",
                                  name=f"pm{c}")
                    pms[c] = pm
                    if c in evict:
                        wv = AP(wrep_ap.tensor, wrep_ap.offset,
                                [wrep_ap.ap[0], [0, nbl]] + list(wrep_ap.ap[1:]))
                        pr = prawp.tile([128, nbl * DIM], bf16, tag="pr",
                                        name=f"pr{c}")
                        nc.scalar.copy(pr[:], pc[:])
                        meng = nc.gpsimd if c in pool_mult else nc.vector
                        meng.tensor_tensor(
                            out=pm[:].rearrange("p (r d) -> p r d", r=nbl),
                            in0=pr[:].rearrange("p (r d) -> p r d", r=nbl),
                            in1=wv, op=ALU.mult)
                    else:
                        hb = nbl // 2
                        wv = AP(wrep_ap.tensor, wrep_ap.offset,
                                [wrep_ap.ap[0], [0, hb]] + list(wrep_ap.ap[1:]))
                        for hh in range(2):
                            hsl = slice(hb * DIM * hh, hb * DIM * (hh + 1))
                            nc.vector.tensor_tensor(
                                out=pm[:, hsl].rearrange("p (r d) -> p r d",
                                                         r=hb),
                                in0=pc[:, hsl].rearrange("p (r d) -> p r d",
                                                         r=hb),
                                in1=wv, op=ALU.mult)

                def emit_sel(c):
                    # hT[d, i] = out0T[d, i]  (identity matmul seeds the
                    #            accumulation group)
                    #          + sum_(il,j) pm[(il,j), d] * seln[(il,j), i]
                    pm = pms.pop(c)
                    for b8 in range(chunks[c]):
                        b = cum[c] + b8
                        for dh in range(2):
                            osl = slice(128 * dh + 4 * b, 128 * dh + 4 * b + 4)
                            nc.tensor.matmul(
                                hacc[:, osl], idn_t,
                                out0_t[:, 128 * dh + 4 * b:128 * dh + 4 * b + 4],
                                start=True, stop=False,
                                skip_group_check=True)
                            nc.tensor.matmul(
                                hacc[:, osl],
                                pm[:, DIM * b8 + 128 * dh:DIM * b8 + 128 * dh + 128],
                                seln_t[:, 4 * b:4 * b + 4],
                                start=False, stop=True,
                                skip_group_check=True)

                # ---- tail pieces, per i-range [ilo, ihi).  silu -> y-matmul
                # -> silu(+bias) on Act/PE; the fin STT + store are emitted
                # separately (late) so they never block the DVE mult queue.
                h1 = work.tile([128, DIM], bf16, tag="h1")
                zz = work.tile([128, DIM], f32, tag="zz")
                fin = work.tile([128, DIM], bf16, tag="fin")
                out_v = out.rearrange("(dh p) i -> p dh i", dh=2)

                def irv(t, ilo, ihi):
                    ap = t[:] if not isinstance(t, AP) else t
                    return ap.rearrange("p (dh i) -> p dh i", dh=2)[
                        :, :, ilo:ihi]

                def tp_silu1(ilo, ihi):
                    nc.scalar.activation(irv(h1, ilo, ihi),
                                         irv(hacc, ilo, ihi), AF.Silu)

                def tp_y(ilo, ihi):
                    for dp in range(2):
                        osl = slice(128 * dp + ilo, 128 * dp + ihi)
                        for kh in range(2):
                            nc.tensor.matmul(
                                yps[:, osl],
                                w2q_t(kh)[:, 128 * dp:128 * (dp + 1)],
                                h1[:, 128 * kh + ilo:128 * kh + ihi],
                                start=(kh == 0), stop=(kh == 1),
                                skip_group_check=True)

                def tp_silu2(ilo, ihi):
                    # b_out2 is all-zero for this problem's inputs (the
                    # build flips to the biased two-op path if the actual
                    # inputs ever carry a nonzero b_out2).
                    if not use_b2:
                        nc.scalar.activation(irv(zz, ilo, ihi),
                                             irv(yps, ilo, ihi), AF.Silu)
                        return
                    for dp in range(2):
                        osl = slice(128 * dp + ilo, 128 * dp + ihi)
                        nc.scalar.activation(zz[:, osl], yps[:, osl],
                                             AF.Silu, bias=b2_col[dp])

                def tail_store(ilo, ihi, per_dh=False):
                    if not per_dh:
                        nc.vector.scalar_tensor_tensor(
                            out=irv(fin, ilo, ihi), in0=irv(zz, ilo, ihi),
                            scalar=K2, in1=irv(elecT_t, ilo, ihi),
                            op0=ALU.mult, op1=ALU.add)
                        nc.sync.dma_start(out=out_v[:, :, ilo:ihi],
                                          in_=irv(fin, ilo, ihi))
                        return
                    # closing piece: silu2 + fin + store per d-half so the
                    # first store's DMA chain overlaps the second half's
                    # compute (tp_silu2 is skipped for this piece)
                    for dp in range(2):
                        osl = slice(128 * dp + ilo, 128 * dp + ihi)
                        if use_b2:
                            nc.scalar.activation(zz[:, osl], yps[:, osl],
                                                 AF.Silu, bias=b2_col[dp])
                        else:
                            nc.scalar.activation(zz[:, osl], yps[:, osl],
                                                 AF.Silu)
                        nc.vector.scalar_tensor_tensor(
                            out=irv(fin, ilo, ihi)[:, dp:dp + 1],
                            in0=irv(zz, ilo, ihi)[:, dp:dp + 1], scalar=K2,
                            in1=irv(elecT_t, ilo, ihi)[:, dp:dp + 1],
                            op0=ALU.mult, op1=ALU.add)
                        nc.sync.dma_start(
                            out=out_v[:, dp:dp + 1, ilo:ihi],
                            in_=irv(fin, ilo, ihi)[:, dp:dp + 1])

                # i-ranges triggered when their last contributing chunk's
                # selector lands (4 blocks -> 16 electrons).  The first piece
                # runs its full silu1/y/silu2 chain mid-loop; the trailing
                # pieces are phase-interleaved (all silu1s as their selectors
                # land, then the y/silu2/store phases back-to-back) so they
                # pipeline through the in-order Act queue instead of
                # laddering.
                half_c = next(i for i in range(nchunk)
                              if cum[i + 1] == NB // 2)
                if tail_mode == 2 and cum[-4:] == (26, 28, 30, 32):
                    # three pieces: (0,64) mid-loop, (64,96) after sel(c7)
                    # in the trailing section, (96,128) closing
                    triggers = {half_c: (0, 64), nchunk - 1: [(96, 128)]}
                    mid2 = (64, 96)
                else:
                    triggers = {half_c: (0, 64), nchunk - 1: [(64, 128)]}
                    mid2 = None
                first_t = min(triggers)
                late = [p for t in sorted(triggers) if t != first_t
                        for p in triggers[t]]

                pool_mult = (frozenset({6, 8}) if tail_mode == 5
                             else frozenset())
                for c in range(nchunk):
                    pc = p_mms(c)
                    if stage >= 4:
                        emit_wmult(c, pc, pool_mult)
                    if stage >= 5 and c >= SEL_LAG:
                        emit_sel(c - SEL_LAG)
                        if (tail_mode == 0 and stage >= 6
                                and c - SEL_LAG == first_t):
                            tp_silu1(*triggers[first_t])
                            tp_y(*triggers[first_t])
                            tp_silu2(*triggers[first_t])
                # tail_mode 4: the first piece's ops would otherwise sit in
                # the fully-saturated Act evict chain; emit them dead last so
                # the closing piece owns the queue heads and the first piece
                # fills idle slots via the out-of-order wait queues.
                if stage >= 6 and tail_mode == 3:
                    pass  # first piece emitted in the trailing section
                if stage >= 5:
                    for c in range(max(nchunk - SEL_LAG, 0), nchunk):
                        emit_sel(c)
                        if (stage >= 6 and mid2 is not None
                                and c == nchunk - 3):
                            tp_silu1(*mid2)
                            tp_y(*mid2)
                            tp_silu2(*mid2)
                        if (tail_mode == 3 and stage >= 6
                                and c == nchunk - SEL_LAG):
                            # first piece's chain, after all evict emissions
                            tp_silu1(*triggers[first_t])
                            tp_y(*triggers[first_t])
                            tp_silu2(*triggers[first_t])
                        if (tail_mode in (0, 3, 4) and stage >= 6
                                and c in triggers and c != first_t):
                            for p in triggers[c]:
                                tp_silu1(*p)
                if stage >= 6:
                    if tail_mode == 2 and mid2 is not None:
                        for p in late:
                            tp_silu1(*p)
                            tp_y(*p)
                        tail_store(*triggers[first_t])
                        tail_store(*mid2)
                        for p in late:
                            tail_store(*p, per_dh=True)
                    elif tail_mode == 4:
                        for p in late:
                            tp_y(*p)
                        for p in late[:-1]:
                            tp_silu2(*p)
                            tail_store(*p)
                        tail_store(*late[-1], per_dh=True)
                        t0 = triggers[first_t]
                        tp_silu1(*t0)
                        tp_y(*t0)
                        tp_silu2(*t0)
                        tail_store(*t0)
                    elif tail_mode in (0, 3):
                        for p in late:
                            tp_y(*p)
                        for p in late[:-1]:
                            tp_silu2(*p)
                        tail_store(*triggers[first_t])
                        for i, p in enumerate(late):
                            tail_store(*p, per_dh=(i == len(late) - 1))
                    else:
                        t0 = triggers[first_t]
                        tp_silu1(*t0)
                        tp_y(*t0)
                        for p in late:
                            tp_silu1(*p)
                        for p in late:
                            tp_y(*p)
                        tp_silu2(*t0)
                        for p in late:
                            tp_silu2(*p)
                        tail_store(*t0)
                        for p in late:
                            tail_store(*p)
                    return

                dummy = work.tile([128, DIM], bf16, tag="fin", name="dummy")
                nc.vector.tensor_copy(dummy[:, 0:128], seln_t)
                nc.sync.dma_start(out=out[0:128, :], in_=dummy[:, 0:128])
                nc.sync.dma_start(out=out[128:256, :], in_=dummy[:, 0:128])

            if reps is None:
                body()
            else:
                with tc.For_i(0, reps, 1):
                    body()
    nc.compile()
    return nc


def _prep_in_maps(inputs):
    elec_emb = np.asarray(inputs["elec_emb"], np.float32)
    up_inp = np.asarray(inputs["up_inp"], np.float32)
    down_inp = np.asarray(inputs["down_inp"], np.float32)
    edge_emb = np.asarray(inputs["edge_emb"], np.float32)
    norm = np.asarray(inputs["norm"], np.float32)
    W_out = np.asarray(inputs["W_out"], np.float32)
    b_out = np.asarray(inputs["b_out"], np.float32)
    W_edge = np.asarray(inputs["W_edge"], np.float32)
    W_out2 = np.asarray(inputs["W_out2"], np.float32)
    b_out2 = np.asarray(inputs["b_out2"], np.float32)
    s1 = float(np.asarray(inputs["scale1"]))
    s2 = float(np.asarray(inputs["scale2"]))
    n_up = int(inputs["n_up"])

    # shared across cores (kh-major halves stacked along columns).
    # elecT carries 1/sqrt2 (it doubles as the residual); w2q carries the
    # silu GAIN; out0 = (elec @ W_out + b_out) * s2 is precomputed here
    # (1.2% of the kernel's FLOPs) and shipped d-major.
    w2q = np.ascontiguousarray(W_out2 * GAIN).astype(BF16)            # [256, 256]
    tq_by_spin = {True: up_inp.astype(BF16), False: down_inp.astype(BF16)}
    wrep = np.tile(W_edge, (4, 1)).astype(BF16)                       # [128, 256]
    idn = np.eye(128, dtype=np.float32)
    out0_full = (elec_emb @ W_out + b_out) * np.float32(s2)           # [1024, 256]
    norm_eff = norm * (s1 * s2)
    il_of_p = (np.arange(128) // 32)

    def halves(a):  # [256, f] -> [128, 2f] kh-major
        return np.concatenate([a[0:128], a[128:256]], axis=1)

    in_maps = []
    for c in range(N_CORES):
        i_lo = c * NI
        is_up = (i_lo + NI) <= n_up  # all electrons in this core share spin
        E = edge_emb[c * NE:(c + 1) * NE].reshape(NI, N_NUC, EDIM)
        # ebf[kh, p, b, il, j] = E[4b+il, 128kh+p, j]; tq is prepended as
        # the first _EDGE0 columns (row k = kh*128+p matches T's rows).
        ebf = np.ascontiguousarray(np.concatenate([
            np.asarray(tq_by_spin[is_up]).reshape(2 * 128, _EDGE0),
            E.reshape(NB, 4, 2, 128, EDIM).transpose(2, 3, 0, 1, 4)
            .reshape(2 * 128, NB * 128).astype(BF16)], axis=1))
        sel = np.zeros((128, NI), np.float32)
        ne_c = norm_eff[i_lo:i_lo + NI]
        for col in range(NI):
            sel[il_of_p == (col % 4), col] = ne_c[col]
        el = elec_emb[i_lo:i_lo + NI]                                 # [128, 256]
        elT = np.ascontiguousarray(el.T) * np.float32(INV_SQRT2)      # [256, 128]
        o0T = np.ascontiguousarray(out0_full[i_lo:i_lo + NI].T)       # [256, 128]
        b2c = np.stack([b_out2[0:128], b_out2[128:256]], axis=1)      # [128, 2]
        cb_arr = np.concatenate([
            wrep, sel.astype(BF16),
            idn.astype(BF16), halves(o0T).astype(BF16),
            halves(elT).astype(BF16), halves(w2q), b2c.astype(BF16)], axis=1)
        assert cb_arr.shape == (128, _CB_COLS), cb_arr.shape
        in_maps.append({
            "edges": ebf,
            "cb": np.ascontiguousarray(cb_arr),
        })
    return in_maps


def _get_runner(use_b2=False):
    global _RUNNER
    if not isinstance(_RUNNER, dict):
        _RUNNER = {}
    if use_b2 not in _RUNNER:
        import jax
        import concourse.mybir as mybir
        from jax.sharding import Mesh, PartitionSpec, NamedSharding
        from jax.experimental.shard_map import shard_map
        from concourse.bass2jax import (_bass_exec_p, install_neuronx_cc_hook,
                                        partition_id_tensor)

        nc = _build_nc(use_b2=use_b2)
        install_neuronx_cc_hook()
        partition_name = (nc.partition_id_tensor.name
                          if nc.partition_id_tensor else None)
        in_names, out_names, out_avals = [], [], []
        for alloc in nc.m.functions[0].allocations:
            if not isinstance(alloc, mybir.MemoryLocationSet):
                continue
            name = alloc.memorylocations[0].name
            if alloc.kind == "ExternalInput":
                if name != partition_name:
                    in_names.append(name)
            elif alloc.kind == "ExternalOutput":
                out_names.append(name)
                out_avals.append(jax.core.ShapedArray(
                    tuple(alloc.tensor_shape), mybir.dt.np(alloc.dtype)))
        n_params = len(in_names)
        all_in = list(in_names) + list(out_names)
        if partition_name is not None:
            all_in.append(partition_name)

        def _body(*args):
            operands = list(args)
            if partition_name is not None:
                operands.append(partition_id_tensor())
            return tuple(_bass_exec_p.bind(
                *operands, out_avals=tuple(out_avals), in_names=tuple(all_in),
                out_names=tuple(out_names), lowering_input_output_aliases=(),
                sim_require_finite=False, sim_require_nnan=False, nc=nc))

        devices = jax.devices()[:N_CORES]
        mesh = Mesh(np.asarray(devices), ("core",))
        n_outs = len(out_avals)
        fn = jax.jit(shard_map(_body, mesh=mesh,
                               in_specs=(PartitionSpec("core"),) * (n_params + n_outs),
                               out_specs=(PartitionSpec("core"),) * n_outs,
                               check_rep=False), keep_unused=True)
        sh = NamedSharding(mesh, PartitionSpec("core"))
        zero_outs = [np.zeros((N_CORES * a.shape[0], *a.shape[1:]), a.dtype)
                     for a in out_avals]

        def run(in_maps):
            per_core = [[np.asarray(m[n]) for n in in_names] for m in in_maps]
            concat_in = [np.concatenate([per_core[c][i] for c in range(N_CORES)],
                                        axis=0) for i in range(n_params)]
            args = [jax.device_put(a, sh) for a in concat_in + zero_outs]
            outs = fn(*args)
            jax.block_until_ready(outs)
            o = np.asarray(outs[out_names.index("out")]).astype(np.float32)
            return o.reshape(N_CORES, DIM, NI)

        _RUNNER[use_b2] = run
    return _RUNNER[use_b2]


def kernel(**inputs) -> np.ndarray:
    use_b2 = bool(np.any(np.asarray(inputs["b_out2"])))
    run = _get_runner(use_b2)
    in_maps = _prep_in_maps(inputs)
    per_core = run(in_maps)                     # [8, 256 d, 128 i]
    return np.ascontiguousarray(
        per_core.transpose(0, 2, 1).reshape(N_EL, DIM))
